# revision 1
# baseline (speedup 1.0000x reference)
"""GATv2 link predictor on 8 TRN2 NeuronCores (Bass/Tile).

Sharding: edges are assigned to the core that owns their dst node
(contiguous N/8-node ranges).  Per core the edge list is bucketed by
(src-half, dst-block) so every 128-edge chunk has all dsts inside one
128-aligned node block of the core's range and all srcs inside one half of
the node table (so gather indices fit int16).  The single per-edge random
access is a dma_gather of the projected source row; the segment softmax and
weighted scatter-add are dense PE matmuls against one-hot selection
matrices built on-chip with is_equal vs iota, accumulated into an
SBUF-resident per-block accumulator.  The softmax division is applied per
node after aggregation: out[v] = (sum ex*xl[src]) / (sum ex).  One
AllGather per layer exchanges projected node tables; decode dot products
use two more gathers with pairs bucketed by (a-half, b-half).
"""

import math
import numpy as np

N = 50000
E = 800000
EL = 100000
IN = 128
HID = 32
HEADS = 4
OUT = 64
NEG_SLOPE = 0.2
LN_EPS = 1e-5
NCORES = 8

R = N // NCORES          # rows per rank (6250)
HALF = N // 2            # src half split (25000)
NB = math.ceil(R / 128)  # dst blocks per rank (49)
RLAST = R - (NB - 1) * 128   # rows in last block (106)
TB = 8                   # chunks per gather tile (1024 idxs)
SENT = 300.0             # dst sentinel (never matches iota 0..127)

F32 = np.float32

_CACHE = {}


def configure(n=50000, e=800000, el=100000, ncores=8):
    """Override problem dims (testing at small scale)."""
    global N, E, EL, NCORES, R, HALF, NB, RLAST
    N, E, EL, NCORES = n, e, el, ncores
    R = N // NCORES
    HALF = N // 2
    NB = math.ceil(R / 128)
    RLAST = R - (NB - 1) * 128
    _CACHE.clear()


# ----------------------------------------------------------------- host prep
def _wrap_idx(idx16):
    """[n] int16 -> dma_gather wrapped layout [128, n/16]."""
    n = idx16.shape[0]
    return np.tile(idx16.reshape(n // 16, 16).T, (8, 1)).copy()


def build_edge_schedule(src, dst):
    """Shared chunk schedule (list of dst-block ids per pool) + per-rank
    slot arrays (int16 gather idx, f32 dst-mod columns)."""
    rank = dst // R
    pool = (src >= HALF).astype(np.int64)
    blk = (dst % R) // 128

    counts = np.zeros((NCORES, 2, NB), np.int64)
    np.add.at(counts, (rank, pool, blk), 1)
    cmax = counts.max(axis=0)                      # [2, NB]
    nchunks = (cmax + 127) // 128                  # chunks needed per (p, B)

    sched = []
    for p in range(2):
        blocks = []
        for b in range(NB):
            blocks += [b] * int(nchunks[p, b])
        while len(blocks) % TB:
            blocks.append(NB - 1)
        sched.append(blocks)

    order = np.lexsort((src, dst))
    src_s, dst_s = src[order], dst[order]
    rank_s, pool_s, blk_s = rank[order], pool[order], blk[order]
    dstm_s = (dst_s % R) % 128

    slot_of_block = []
    for p in range(2):
        sob = {}
        for ci, b in enumerate(sched[p]):
            sob.setdefault(b, []).append(ci)
        slot_of_block.append(sob)

    per_rank = []
    for r in range(NCORES):
        m = rank_s == r
        rsrc, rpool, rblk, rdstm = src_s[m], pool_s[m], blk_s[m], dstm_s[m]
        entry = {"idx": [], "dstc": []}
        for p in range(2):
            n_slots = len(sched[p]) * 128
            s_src = np.zeros(n_slots, np.int16)
            s_dst = np.full(n_slots, SENT, F32)
            mp = rpool == p
            psrc, pblk, pdstm = rsrc[mp], rblk[mp], rdstm[mp]
            for b in range(NB):
                mb = pblk == b
                nb_e = int(mb.sum())
                if nb_e == 0:
                    continue
                chunks = slot_of_block[p][b]
                base = np.concatenate(
                    [np.arange(ci * 128, ci * 128 + 128) for ci in chunks])
                tgt = base[:nb_e]
                s_src[tgt] = (psrc[mb] - p * HALF).astype(np.int16)
                s_dst[tgt] = pdstm[mb].astype(F32)
            entry["idx"].append(_wrap_idx(s_src))
            entry["dstc"].append(np.ascontiguousarray(
                s_dst.reshape(-1, 128).T))      # [128, n_chunks]
        per_rank.append(entry)
    return {"sched": sched, "per_rank": per_rank}


def build_decode_schedule(eli):
    npairs = EL // NCORES
    gmax = np.zeros(4, np.int64)
    parts = []
    for r in range(NCORES):
        a = eli[0, r * npairs:(r + 1) * npairs]
        b = eli[1, r * npairs:(r + 1) * npairs]
        g = (a >= HALF) * 2 + (b >= HALF)
        parts.append((a, b, g))
        for k in range(4):
            gmax[k] = max(gmax[k], int((g == k).sum()))
    gslots = [(int(v) + TB * 128 - 1) // (TB * 128) * (TB * 128) for v in gmax]
    per_rank = []
    for r in range(NCORES):
        a, b, g = parts[r]
        ia, ib = [], []
        perm = np.full(sum(gslots), -1, np.int64)
        off = 0
        for k in range(4):
            mk = g == k
            nk = int(mk.sum())
            sa = np.zeros(gslots[k], np.int16)
            sb = np.zeros(gslots[k], np.int16)
            sa[:nk] = (a[mk] - (k // 2) * HALF).astype(np.int16)
            sb[:nk] = (b[mk] - (k % 2) * HALF).astype(np.int16)
            perm[off:off + nk] = np.nonzero(mk)[0]
            ia.append(_wrap_idx(sa))
            ib.append(_wrap_idx(sb))
            off += gslots[k]
        per_rank.append({"ia": np.concatenate(ia, axis=1),
                         "ib": np.concatenate(ib, axis=1), "perm": perm})
    return {"gslots": gslots, "per_rank": per_rank, "npairs": npairs}


def host_prep(inputs):
    ei = np.asarray(inputs["edge_index"])
    loops = np.arange(N, dtype=ei.dtype)
    src = np.concatenate([ei[0], loops]).astype(np.int64)
    dst = np.concatenate([ei[1], loops]).astype(np.int64)
    es = build_edge_schedule(src, dst)
    ds = build_decode_schedule(np.asarray(inputs["edge_label_index"]))

    x = np.ascontiguousarray(np.asarray(inputs["x"], F32))
    att1 = np.asarray(inputs["att1"], F32)

    shared = {
        "x": x,
        "W1l": np.ascontiguousarray(inputs["W1l"], F32),
        "W1r": np.ascontiguousarray(inputs["W1r"], F32),
        "b1l": np.tile(np.asarray(inputs["b1l"], F32).reshape(1, -1), (128, 1)),
        "b1r": np.tile(np.asarray(inputs["b1r"], F32).reshape(1, -1), (128, 1)),
        "attr1": np.tile(att1.reshape(1, -1), (128, 1)),
        "attr2": np.tile(np.asarray(inputs["att2"], F32).reshape(1, -1),
                         (128, 1)),
        "bias1": np.tile(np.asarray(inputs["bias1"], F32).reshape(1, -1), (128, 1)),
        "g1": np.tile(np.asarray(inputs["g1"], F32).reshape(1, -1), (128, 1)),
        "be1": np.tile(np.asarray(inputs["be1"], F32).reshape(1, -1), (128, 1)),
        "W2l": np.ascontiguousarray(inputs["W2l"], F32),
        "W2r": np.ascontiguousarray(inputs["W2r"], F32),
        "b2l": np.tile(np.asarray(inputs["b2l"], F32).reshape(1, -1), (128, 1)),
        "b2r": np.tile(np.asarray(inputs["b2r"], F32).reshape(1, -1), (128, 1)),
        "bias2": np.tile(np.asarray(inputs["bias2"], F32).reshape(1, -1), (128, 1)),
        "g2": np.tile(np.asarray(inputs["g2"], F32).reshape(1, -1), (128, 1)),
        "be2": np.tile(np.asarray(inputs["be2"], F32).reshape(1, -1), (128, 1)),
        "ident": np.eye(128, dtype=F32),
        "iota_c": np.arange(128, dtype=F32).reshape(128, 1),
        "iota_r": np.tile(np.arange(128, dtype=F32).reshape(1, 128), (128, 1)),
        "ones11": np.ones((1, 1), F32),
    }
    in_maps = []
    for r in range(NCORES):
        m = dict(shared)
        m["x_own"] = np.ascontiguousarray(x[r * R:(r + 1) * R])
        pr = es["per_rank"][r]
        m["idx0"], m["idx1"] = pr["idx"][0], pr["idx"][1]
        m["dstc0"], m["dstc1"] = pr["dstc"][0], pr["dstc"][1]
        m["dstr0"] = np.ascontiguousarray(pr["dstc"][0].T.reshape(1, -1))
        m["dstr1"] = np.ascontiguousarray(pr["dstc"][1].T.reshape(1, -1))
        dr = ds["per_rank"][r]
        m["dia"], m["dib"] = dr["ia"], dr["ib"]
        in_maps.append(m)
    return {"es": es, "ds": ds}, in_maps


# ------------------------------------------------------------- kernel builder
def build_kernel(meta):
    import concourse.bacc as bacc
    import concourse.bass as bass
    import concourse.mybir as mybir
    import concourse.tile as tile

    dt = mybir.dt
    AF = mybir.ActivationFunctionType
    OP = mybir.AluOpType
    AX = mybir.AxisListType
    AP = bass.AP

    sched = meta["es"]["sched"]
    gslots = meta["ds"]["gslots"]
    n_dec_slots = sum(gslots)
    n_dec_tiles = n_dec_slots // (TB * 128)
    # group (z-half pair) of each decode tile
    dec_tile_group = []
    off = 0
    for k in range(4):
        for _ in range(gslots[k] // (TB * 128)):
            dec_tile_group.append(k)

    nch = [len(sched[p]) for p in range(2)]
    nt = [nch[p] // TB for p in range(2)]

    nc = bacc.Bacc("TRN2", target_bir_lowering=False, debug=False,
                   num_devices=NCORES)

    def din(name, shape, d=dt.float32):
        return nc.dram_tensor(name, list(shape), d, kind="ExternalInput")

    x_in = din("x", [N, IN])
    x_own_in = din("x_own", [R, IN])
    W1l, W1r = din("W1l", [IN, IN]), din("W1r", [IN, IN])
    W2l, W2r = din("W2l", [IN, OUT]), din("W2r", [IN, OUT])
    ident_in, iota_c_in = din("ident", [128, 128]), din("iota_c", [128, 1])
    iota_r_in, ones11_in = din("iota_r", [128, 128]), din("ones11", [1, 1])
    row_ins = {nm: din(nm, [128, w]) for nm, w in [
        ("b1l", IN), ("b1r", IN), ("bias1", IN), ("g1", IN), ("be1", IN),
        ("b2l", OUT), ("b2r", OUT), ("bias2", OUT), ("g2", OUT), ("be2", OUT),
        ("attr1", IN), ("attr2", OUT)]}
    idx_in = [din(f"idx{p}", [128, nch[p] * 8], dt.int16) for p in range(2)]
    dstc_in = [din(f"dstc{p}", [128, nch[p]]) for p in range(2)]
    dstr_in = [din(f"dstr{p}", [1, nch[p] * 128]) for p in range(2)]
    dia_in = din("dia", [128, n_dec_slots // 16], dt.int16)
    dib_in = din("dib", [128, n_dec_slots // 16], dt.int16)

    out_dec = nc.dram_tensor("out_dec", [n_dec_slots], dt.float32,
                             kind="ExternalOutput")

    xl_dram = nc.dram_tensor("xl_tab", [N, IN], dt.float32)
    ag1_in = nc.dram_tensor("ag1_in", [R, 128], dt.float32)
    hlhr = nc.dram_tensor("hlhr", [N, 128], dt.float32, addr_space="Shared")
    ag2_in = nc.dram_tensor("ag2_in", [R, OUT], dt.float32)
    z_full = nc.dram_tensor("z_full", [N, OUT], dt.float32, addr_space="Shared")

    NXB = math.ceil(N / 128)

    def bcast_row(ap, nb_, w):
        """[128, w] replicated row -> [128, nb_, w] view (mid dim bcast)."""
        return AP(ap.tensor, ap.offset, [[ap.ap[0][0], 128], [0, nb_], [1, w]])

    from contextlib import ExitStack
    with tile.TileContext(nc) as tc, ExitStack() as stack:
        pp = stack.enter_context(tc.tile_pool(name="persist", bufs=1))
        ident = pp.tile([128, 128], dt.float32)
        nc.sync.dma_start(ident[:], ident_in[:])
        iota_c = pp.tile([128, 1], dt.float32)
        nc.sync.dma_start(iota_c[:], iota_c_in[:])
        iota_r = pp.tile([128, 128], dt.float32)
        nc.sync.dma_start(iota_r[:], iota_r_in[:])
        ones11 = pp.tile([1, 1], dt.float32)
        nc.sync.dma_start(ones11[:], ones11_in[:])
        w1l_sb = pp.tile([IN, IN], dt.float32)
        nc.sync.dma_start(w1l_sb[:], W1l[:])
        w1r_sb = pp.tile([IN, IN], dt.float32)
        nc.sync.dma_start(w1r_sb[:], W1r[:])
        w2l_sb = pp.tile([IN, OUT], dt.float32)
        nc.sync.dma_start(w2l_sb[:], W2l[:])
        w2r_sb = pp.tile([IN, OUT], dt.float32)
        nc.sync.dma_start(w2r_sb[:], W2r[:])
        rows = {}
        for nm, t in row_ins.items():
            rows[nm] = pp.tile(list(t.shape), dt.float32, name=f"row_{nm}", tag=f"row_{nm}")
            nc.sync.dma_start(rows[nm][:], t[:])

        idx_sb = []
        dstc_sb = []
        for p in range(2):
            it = pp.tile([128, nch[p] * 8], dt.int16, name=f"idx{p}", tag=f"idx{p}")
            nc.sync.dma_start(it[:], idx_in[p][:])
            idx_sb.append(it)
            ct = pp.tile([128, nch[p]], dt.float32, name=f"dstc{p}", tag=f"dstc{p}")
            nc.sync.dma_start(ct[:], dstc_in[p][:])
            dstc_sb.append(ct)
        dia_sb = pp.tile([128, n_dec_slots // 16], dt.int16)
        nc.sync.dma_start(dia_sb[:], dia_in[:])
        dib_sb = pp.tile([128, n_dec_slots // 16], dt.int16)
        nc.sync.dma_start(dib_sb[:], dib_in[:])

        xr_own = pp.tile([128, NB, IN], dt.bfloat16)
        acc1 = pp.tile([128, NB, IN + HEADS], dt.float32)
        nc.vector.memset(acc1[:], 0.0)
        hr_own = pp.tile([128, NB, OUT], dt.bfloat16)
        acc2 = pp.tile([128, NB, OUT + 1], dt.float32)
        nc.vector.memset(acc2[:], 0.0)
        out_sb = pp.tile([128, n_dec_slots // 128], dt.float32)

        # ---------------- phase A: projections (xl table + own xr) -------
        with tc.tile_pool(name="pA", bufs=3) as rp, \
                tc.tile_pool(name="pAp", bufs=2, space="PSUM") as ps:
            for nb_ in range(NXB):
                lo = nb_ * 128
                cnt = min(128, N - lo)
                xb = rp.tile([128, IN], dt.float32, tag="xb")
                if cnt < 128:
                    nc.vector.memset(xb[:], 0.0)
                nc.sync.dma_start(xb[:cnt], x_in[lo:lo + cnt, :])
                xT_ps = ps.tile([128, 128], dt.float32, tag="xT")
                nc.tensor.transpose(xT_ps[:], xb[:], ident[:])
                xT = rp.tile([128, 128], dt.float32, tag="xTs")
                nc.vector.tensor_copy(xT[:], xT_ps[:])
                xl_ps = ps.tile([128, IN], dt.float32, tag="xl")
                nc.tensor.matmul(xl_ps[:], lhsT=xT[:], rhs=w1l_sb[:],
                                 start=True, stop=True)
                xls = rp.tile([128, IN], dt.float32, tag="xls")
                nc.vector.tensor_tensor(
                    xls[:], xl_ps[:],
                    rows["b1l"][:], op=OP.add)
                nc.sync.dma_start(xl_dram[lo:lo + cnt, :], xls[:cnt])
            for B in range(NB):
                lo = B * 128
                cnt = min(128, R - lo)
                xb = rp.tile([128, IN], dt.float32, tag="xb2")
                if cnt < 128:
                    nc.vector.memset(xb[:], 0.0)
                nc.sync.dma_start(xb[:cnt], x_own_in[lo:lo + cnt, :])
                xT_ps = ps.tile([128, 128], dt.float32, tag="xT")
                nc.tensor.transpose(xT_ps[:], xb[:], ident[:])
                xT = rp.tile([128, 128], dt.float32, tag="xTs")
                nc.vector.tensor_copy(xT[:], xT_ps[:])
                xr_ps = ps.tile([128, IN], dt.float32, tag="xl")
                nc.tensor.matmul(xr_ps[:], lhsT=xT[:], rhs=w1r_sb[:],
                                 start=True, stop=True)
                nc.vector.tensor_tensor(
                    xr_own[:, B, :], xr_ps[:],
                    rows["b1r"][:], op=OP.add)

        # ---------------- edge aggregation (shared for both layers) ------
        def edge_phase(feat, nheads, gsrc_tab, gsrc_cols, side_own, a_row,
                       accum):
            """feat: per-edge feature width; nheads: head count.
            gsrc_tab: DRAM table rows [N, gsrc_cols]; gathers first `feat`
            cols.  side_own: SBUF [128, NB, feat] dst-side rows.  a_mat: SBUF
            [feat, nheads] logit weights.  accum: SBUF [128, NB, feat+nheads].
            """
            with tc.tile_pool(name=f"pE{feat}", bufs=2) as rp, \
                    tc.tile_pool(name=f"pEp{feat}", bufs=2, space="PSUM") as ps:
                for p in range(2):
                    for t in range(nt[p]):
                        xlg = rp.tile([128, TB, feat], dt.float32, tag="xlg")
                        nc.gpsimd.dma_gather(
                            xlg[:],
                            gsrc_tab[p * HALF:(p + 1) * HALF, 0:feat],
                            idx_sb[p][:, t * TB * 8:(t + 1) * TB * 8],
                            TB * 128, TB * 128, feat,
                            elem_step=gsrc_cols, single_packet=False)
                        gc = rp.tile([128, TB, 128], dt.bfloat16, tag="gc")
                        dc = dstc_sb[p][:]
                        nc.vector.tensor_tensor(
                            gc[:],
                            AP(dc.tensor, dc.offset + t * TB,
                               [[dc.ap[0][0], 128], [1, TB], [0, 128]]),
                            AP(iota_r[:].tensor, iota_r[:].offset,
                               [[iota_r[:].ap[0][0], 128], [0, TB],
                                [1, 128]]),
                            op=OP.is_equal)
                        dstr_rep = rp.tile([128, TB * 128], dt.float32,
                                           tag="dstr_rep")
                        nc.sync.dma_start(
                            dstr_rep[:],
                            AP(dstr_in[p][:].tensor, t * TB * 128,
                               [[0, 128], [1, TB * 128]]))
                        gtb = rp.tile([128, TB * 128], dt.bfloat16, tag="gtb")
                        nc.vector.tensor_tensor(
                            gtb[:],
                            iota_c[:].to_broadcast([128, TB * 128]),
                            dstr_rep[:], op=OP.is_equal)
                        rv = rp.tile([128, TB, feat], dt.float32, tag="rv")
                        for g in range(TB // 4):
                            vtb = ps.tile([128, 4, feat], dt.float32,
                                          tag="vtb")
                            for jj in range(4):
                                j = g * 4 + jj
                                B = sched[p][t * TB + j]
                                nc.tensor.matmul(
                                    vtb[:, jj, :],
                                    lhsT=gtb[:, j * 128:(j + 1) * 128],
                                    rhs=side_own[:, B, :],
                                    start=True, stop=True)
                            vs = rp.tile([128, 4, feat], dt.float32,
                                         tag="vs")
                            nc.vector.tensor_add(
                                vs[:], vtb[:], xlg[:, g * 4:(g + 1) * 4, :])
                            sc = rp.tile([128, 4, feat], dt.float32,
                                         tag="sc")
                            nc.scalar.activation(
                                sc[:].rearrange("p a b -> p (a b)"),
                                vs[:].rearrange("p a b -> p (a b)"),
                                AF.Copy, scale=NEG_SLOPE)
                            nc.vector.tensor_max(
                                rv[:, g * 4:(g + 1) * 4, :], sc[:], vs[:])
                        ch = feat // nheads
                        lm = rp.tile([128, TB, feat], dt.float32, tag="lm")
                        nc.vector.tensor_tensor(
                            lm[:], rv[:],
                            AP(a_row[:].tensor, a_row[:].offset,
                               [[a_row[:].ap[0][0], 128], [0, TB], [1, feat]]),
                            op=OP.mult)
                        lg = rp.tile([128, TB * nheads], dt.float32, tag="lg")
                        nc.vector.tensor_reduce(
                            lg[:], lm[:].rearrange("p t (h c) -> p t h c",
                                                   h=nheads),
                            axis=AX.X, op=OP.add)
                        ex_em = rp.tile([128, TB, nheads], dt.float32,
                                        tag="ex_em")
                        nc.scalar.activation(
                            ex_em[:].rearrange("p a b -> p (a b)"), lg[:],
                            AF.Exp)
                        ebuf = rp.tile([128, TB, feat + nheads], dt.bfloat16,
                                       tag="ebuf")
                        eb = ebuf[:]
                        nc.vector.tensor_tensor(
                            AP(eb.tensor, eb.offset,
                               [[eb.ap[0][0], 128],
                                [feat + nheads, TB], [ch, nheads], [1, ch]]),
                            xlg[:].rearrange("p t (h c) -> p t h c",
                                             h=nheads),
                            ex_em[:, :, :, None].to_broadcast(
                                [128, TB, nheads, ch]),
                            op=OP.mult)
                        nc.vector.tensor_copy(
                            ebuf[:, :, feat:feat + nheads], ex_em[:])
                        tsched = sched[p][t * TB:(t + 1) * TB]
                        oacc = None
                        for j in range(TB):
                            B = tsched[j]
                            first = j == 0 or tsched[j - 1] != B
                            last = j == TB - 1 or tsched[j + 1] != B
                            if first:
                                oacc = ps.tile([128, feat + nheads],
                                               dt.float32, tag="oacc")
                            nc.tensor.matmul(
                                oacc[:], lhsT=gc[:, j, :], rhs=ebuf[:, j, :],
                                start=first, stop=last)
                            if last:
                                nc.vector.tensor_add(
                                    accum[:, B, :], accum[:, B, :], oacc[:])

        edge_phase(IN, HEADS, xl_dram, IN, xr_own, rows["attr1"], acc1)

        # ---------------- phase C: divide + bias + LN + ELU on acc1 ------
        def post_layer(accum, feat, nheads, bias_row, g_row, be_row, elu):
            CB = 7 if NB % 7 == 0 else 1  # blocks per op chunk
            with tc.tile_pool(name=f"pC{feat}", bufs=2) as rp:
                stride = feat + nheads
                for b0 in range(0, NB, CB):
                    nb_ = min(CB, NB - b0)
                    a = accum[:, b0:b0 + nb_, :]
                    rcp = rp.tile([128, CB, nheads], dt.float32, tag="rcp")
                    dn = rp.tile([128, CB, nheads], dt.float32, tag="dn")
                    nc.vector.tensor_scalar(
                        dn[:, :nb_, :], a[:, :, feat:feat + nheads], 1e-16,
                        None, op0=OP.add)
                    nc.vector.reciprocal(rcp[:, :nb_, :], dn[:, :nb_, :])
                    hv = accum[:, b0:b0 + nb_, 0:feat]
                    ch = feat // nheads
                    hv4 = hv.rearrange("p b (h c) -> p b h c", h=nheads)
                    nc.vector.tensor_tensor(
                        hv4, hv4,
                        rcp[:, :nb_, :, None].to_broadcast(
                            [128, nb_, nheads, ch]), op=OP.mult)
                    nc.vector.tensor_tensor(
                        hv, hv, bcast_row(bias_row[:], nb_, feat), op=OP.add)
                    mu = rp.tile([128, CB], dt.float32, tag="mu")
                    nc.vector.tensor_reduce(mu[:, :nb_], hv, axis=AX.X,
                                            op=OP.add)
                    nc.vector.tensor_scalar_mul(mu[:, :nb_], mu[:, :nb_],
                                                1.0 / feat)
                    nc.vector.tensor_tensor(
                        hv, hv,
                        mu[:, :nb_, None].to_broadcast([128, nb_, feat]),
                        op=OP.subtract)
                    var = rp.tile([128, CB], dt.float32, tag="var")
                    sq = rp.tile([128, CB, feat], dt.float32, tag="sq")
                    nc.vector.tensor_tensor(sq[:, :nb_, :], hv, hv,
                                            op=OP.mult)
                    nc.vector.tensor_reduce(var[:, :nb_], sq[:, :nb_, :],
                                            axis=AX.X, op=OP.add)
                    nc.vector.tensor_scalar(var[:, :nb_], var[:, :nb_],
                                            1.0 / feat, LN_EPS, op0=OP.mult,
                                            op1=OP.add)
                    nc.vector.reciprocal(var[:, :nb_], var[:, :nb_])
                    rs = rp.tile([128, CB], dt.float32, tag="rs")
                    nc.scalar.activation(rs[:, :nb_], var[:, :nb_], AF.Sqrt)
                    nc.vector.tensor_tensor(
                        hv, hv,
                        rs[:, :nb_, None].to_broadcast([128, nb_, feat]),
                        op=OP.mult)
                    nc.vector.tensor_tensor(
                        hv, hv, bcast_row(g_row[:], nb_, feat), op=OP.mult)
                    nc.vector.tensor_tensor(
                        hv, hv, bcast_row(be_row[:], nb_, feat), op=OP.add)
                    if elu:
                        mn = rp.tile([128, CB, feat], dt.float32, tag="mn")
                        nc.vector.tensor_scalar(mn[:, :nb_, :], hv, 0.0,
                                                None, op0=OP.min)
                        ex0 = rp.tile([128, CB, feat], dt.float32, tag="ex0")
                        nc.scalar.activation(ex0[:, :nb_, :], mn[:, :nb_, :],
                                             AF.Exp)
                        nc.scalar.activation(hv, hv, AF.Relu)
                        nc.vector.tensor_tensor(hv, hv, ex0[:, :nb_, :],
                                                op=OP.add)
                        nc.vector.tensor_scalar(hv, hv, -1.0, None,
                                                op0=OP.add)

        post_layer(acc1, IN, HEADS, rows["bias1"], rows["g1"], rows["be1"],
                   True)

        # ---------------- phase D: project h -> hl/hr, AllGather ---------
        with tc.tile_pool(name="pD", bufs=3) as rp, \
                tc.tile_pool(name="pDp", bufs=2, space="PSUM") as ps:
            for B in range(NB):
                cnt = 128 if B < NB - 1 else RLAST
                hT_ps = ps.tile([128, 128], dt.float32, tag="hT")
                nc.tensor.transpose(hT_ps[:], acc1[:, B, 0:IN], ident[:])
                hT = rp.tile([128, 128], dt.float32, tag="hTs")
                nc.vector.tensor_copy(hT[:], hT_ps[:])
                pl = ps.tile([128, OUT], dt.float32, tag="pl")
                nc.tensor.matmul(pl[:], lhsT=hT[:], rhs=w2l_sb[:],
                                 start=True, stop=True)
                pr_ = ps.tile([128, OUT], dt.float32, tag="pr")
                nc.tensor.matmul(pr_[:], lhsT=hT[:], rhs=w2r_sb[:],
                                 start=True, stop=True)
                stage = rp.tile([128, 128], dt.float32, tag="stage")
                nc.vector.tensor_tensor(
                    stage[:, 0:OUT], pl[:],
                    rows["b2l"][:], op=OP.add)
                nc.vector.tensor_tensor(
                    stage[:, OUT:128], pr_[:],
                    rows["b2r"][:], op=OP.add)
                nc.vector.tensor_copy(hr_own[:, B, :], stage[:, OUT:128])
                nc.sync.dma_start(ag1_in[B * 128:B * 128 + cnt, :],
                                  stage[:cnt])
        nc.gpsimd.collective_compute(
            "AllGather", OP.bypass,
            replica_groups=[list(range(NCORES))],
            ins=[ag1_in[:]], outs=[hlhr[:]])

        # ---------------- phase E: layer-2 aggregation -------------------
        edge_phase(OUT, 1, hlhr, 128, hr_own, rows["attr2"], acc2)

        post_layer(acc2, OUT, 1, rows["bias2"], rows["g2"], rows["be2"],
                   False)

        # ---------------- phase F: z AllGather ---------------------------
        with tc.tile_pool(name="pF", bufs=3) as rp:
            for B in range(NB):
                cnt = 128 if B < NB - 1 else RLAST
                nc.sync.dma_start(ag2_in[B * 128:B * 128 + cnt, :],
                                  acc2[:cnt, B, 0:OUT])
        nc.gpsimd.collective_compute(
            "AllGather", OP.bypass,
            replica_groups=[list(range(NCORES))],
            ins=[ag2_in[:]], outs=[z_full[:]])

        # ---------------- phase G: decode dot products -------------------
        with tc.tile_pool(name="pG", bufs=3) as rp:
            for t in range(n_dec_tiles):
                k = dec_tile_group[t]
                ka, kb = k // 2, k % 2
                za = rp.tile([128, TB, OUT], dt.float32, tag="za")
                nc.gpsimd.dma_gather(
                    za[:], z_full[ka * HALF:(ka + 1) * HALF, :],
                    dia_sb[:, t * TB * 8:(t + 1) * TB * 8],
                    TB * 128, TB * 128, OUT, single_packet=False)
                zb = rp.tile([128, TB, OUT], dt.float32, tag="zb")
                nc.gpsimd.dma_gather(
                    zb[:], z_full[kb * HALF:(kb + 1) * HALF, :],
                    dib_sb[:, t * TB * 8:(t + 1) * TB * 8],
                    TB * 128, TB * 128, OUT, single_packet=False)
                prod = rp.tile([128, TB, OUT], dt.float32, tag="prod")
                nc.vector.tensor_mul(prod[:], za[:], zb[:])
                nc.vector.tensor_reduce(out_sb[:, t * TB:(t + 1) * TB],
                                        prod[:], axis=AX.X, op=OP.add)
            ods = out_sb[:]
            nc.sync.dma_start(
                AP(out_dec[:].tensor, 0,
                   [[1, 128], [128, n_dec_slots // 128]]),
                ods)

    nc.compile()
    return nc


# ------------------------------------------------------------------ runner
def kernel(_trace=False, **inputs):
    from concourse.bass_utils import run_bass_kernel_spmd

    meta, in_maps = host_prep(inputs)
    key = "k"
    if key not in _CACHE:
        _CACHE[key] = build_kernel(meta)
    nc = _CACHE[key]
    res = run_bass_kernel_spmd(nc, in_maps, list(range(NCORES)),
                               trace=bool(_trace))
    npairs = meta["ds"]["npairs"]
    out = np.zeros(EL, F32)
    for r in range(NCORES):
        od = res.results[r]["out_dec"]
        perm = meta["ds"]["per_rank"][r]["perm"]
        m = perm >= 0
        out[r * npairs + perm[m]] = od[m]
    if _trace:
        return out, res
    return out

